# revision 32
# baseline (speedup 1.0000x reference)
"""MixER MoE-hypernetwork kernel for 8 Trainium2 NeuronCores.

Expert-parallel: core e handles expert e (NEXP == n_cores == 8).

v2 schedule: the Scalar/ACT engine is the hard bottleneck (96 Silu
ACTIVATEs x ~2.05us = ~197us/core minimum), so everything else is arranged
to hide under a continuously-fed ACT queue:
  - H-block loads + deltaT stores grouped (1 DMA per 4-block group) and
    issued from the GpSimd queue; regathers/y/out on Sync. The Scalar queue
    carries ACTIVATEs only.
  - MLP emitted via a lag-gated deepest-first worklist: a stage unit is
    only emitted >=2 ACT-units after the unit producing its input, so the
    in-order engine queues pipeline across envs with no ACT stalls.
  - L4 paired: two envs' final matmuls run concurrently in separate PE
    column tiles into one [128,2048] PSUM tile; single full-width DVE
    gate-multiply epilogue; b4-bias gate term hoisted to the host.
  - per-block PSUM casts in the delta phase free PSUM banks early.
Host: gate softmax, y transpose, H permute/scale/cast to fp8, sum of the 8
per-expert partials + gate-weighted fb4 bias term.
"""
import os
import numpy as np
import ml_dtypes

import concourse.bass as bass
import concourse.bacc as bacc
import concourse.tile as tile
from concourse import mybir
from concourse.bass_utils import run_bass_kernel_spmd

# ---- problem dims (hardcoded; must match the grader's setup_inputs) ----
DATA, WIDTH, CTXD, NEXP, ENVS, NPTS = 64, 256, 128, 8, 16, 2048
SIZES = [WIDTH * DATA, WIDTH, WIDTH * WIDTH, WIDTH, WIDTH * WIDTH, WIDTH,
         DATA * WIDTH, DATA]
OFFS = np.cumsum([0] + SIZES)
BLK = 2048
NBLK = 80                          # weight regions only: 163840 = 80*2048
NETW = NBLK * BLK

# device-layout offsets into the weight-delta stream: [W1T | W2T | W3T | W4T]
O_W1, O_W2, O_W3, O_W4 = 0, 16384, 81920, 147456
NBIAS = 6                          # bias chunks of 128 rows (b1,b1,b2,b2,b3,b3)

F32 = mybir.dt.float32
BF16 = mybir.dt.bfloat16
BF16_NP = ml_dtypes.bfloat16
FP8 = mybir.dt.float8e4
FP8_NP = ml_dtypes.float8_e4m3

N_CORES = 8
TRACE = os.environ.get("MIXER_TRACE", "0") == "1"

if TRACE:
    # The agent image's antenv lacks axon_hooks, so run_bass_kernel_spmd's
    # trace path can't find the NTFF profile hook. Shim it with the ctypes
    # hook factory that trn_boot ships. Profiling-only; inert when TRACE=0.
    try:
        from antenv.axon_hooks import get_axon_ntff_profile_hook  # noqa: F401
    except ImportError:
        import sys as _sys
        import types as _types
        try:
            from trn_agent_boot.trn_boot import _ntff_profile_via_ctypes
            _hook = _ntff_profile_via_ctypes("/opt/axon/libaxon_pjrt.so")
            import antenv as _antenv
            _mod = _types.ModuleType("antenv.axon_hooks")
            _mod.get_axon_ntff_profile_hook = lambda: _hook
            _mod.set_axon_ntff_profile_hook = lambda h: None
            _sys.modules["antenv.axon_hooks"] = _mod
            _antenv.axon_hooks = _mod
        except Exception as _e:  # pragma: no cover - profiling is best-effort
            print(f"NTFF hook shim failed: {_e}")

LAST_RESULTS = None  # BassKernelResults of the most recent run (for test.py)

_NC_CACHE = {}
_PERM_CACHE = {}


# --------------------------------------------------------------------------
# host-side preprocessing
# --------------------------------------------------------------------------
def _build_perm():
    """perm[new_row] = old_row of H's NET axis, weight regions only."""
    if "perm" in _PERM_CACHE:
        return _PERM_CACHE["perm"]
    perm = np.zeros(NETW, dtype=np.int64)
    # W1: orig OFFS[0] + w*DATA+d  -> new O_W1 + d*WIDTH+w   ([64,256] = fW1T)
    d, w = np.meshgrid(np.arange(DATA), np.arange(WIDTH), indexing="ij")
    perm[O_W1 + (d * WIDTH + w).ravel()] = OFFS[0] + (w * DATA + d).ravel()
    # W2/W3: orig + v*WIDTH+w (v,w) -> new + w*WIDTH+v  ([256,256] = fW2T)
    w2, v2 = np.meshgrid(np.arange(WIDTH), np.arange(WIDTH), indexing="ij")
    perm[O_W2 + (w2 * WIDTH + v2).ravel()] = OFFS[2] + (v2 * WIDTH + w2).ravel()
    perm[O_W3 + (w2 * WIDTH + v2).ravel()] = OFFS[4] + (v2 * WIDTH + w2).ravel()
    # W4: orig + d*WIDTH+w (d,w) -> new + w*DATA+d  ([256,64] = fW4T)
    d4, w4 = np.meshgrid(np.arange(DATA), np.arange(WIDTH), indexing="ij")
    perm[O_W4 + (w4 * DATA + d4).ravel()] = OFFS[6] + (d4 * WIDTH + w4).ravel()
    _PERM_CACHE["perm"] = perm
    return perm


def _build_scale(beta_e):
    ib = np.float32(1.0 / beta_e)
    scale = np.ones(NETW, dtype=np.float32)
    scale[O_W2:O_W2 + WIDTH * WIDTH] = ib
    scale[O_W3:O_W3 + WIDTH * WIDTH] = ib
    scale[O_W4:O_W4 + WIDTH * DATA] = ib
    return scale


def _bias_rows():
    """orig H rows for the 6 bias chunks of 128 (b1,b1,b2,b2,b3,b3)."""
    rows = np.zeros(NBIAS * 128, dtype=np.int64)
    rows[0:256] = OFFS[1] + np.arange(WIDTH)          # b1
    rows[256:512] = OFFS[3] + np.arange(WIDTH)        # b2
    rows[512:768] = OFFS[5] + np.arange(WIDTH)        # b3
    return rows


def _prep_inputs(y, ctx, W, b, H, G, beta):
    """Returns (in_maps, host_term): one dict per core + [ENVS, DATA] bias
    term (gate-weighted fb4) to add on the host after summing cores."""
    perm = _build_perm()
    brows = _bias_rows()

    # gate softmax on host (tiny)
    logits = ctx.astype(np.float32) @ G.astype(np.float32).T      # [B, E]
    m = logits.max(-1, keepdims=True)
    eg = np.exp(logits - m)
    gate = (eg / eg.sum(-1, keepdims=True)).astype(np.float32)

    # gate-weighted fb4 term: sum_e gate[b,e] * (b4[e] + H4[e] @ ctx[b])
    fb4 = np.stack([
        b[3][e][None, :] + ctx.astype(np.float32)
        @ H[e][OFFS[7]:OFFS[7] + DATA].astype(np.float32).T
        for e in range(NEXP)], axis=1)                            # [B, E, 64]
    host_term = np.einsum("be,bed->bd", gate, fb4).astype(np.float32)

    yT = np.ascontiguousarray(y.transpose(0, 2, 1)).astype(BF16_NP)
    # ctx^T padded to 32 cols so delta matmuls write full PSUM quadrants
    ctxT = np.zeros((CTXD, 32), dtype=BF16_NP)
    ctxT[:, :ENVS] = ctx.T.astype(BF16_NP)

    in_maps = []
    for e in range(NEXP):
        be = float(beta[e])
        scale = _build_scale(be)
        Hp = H[e][perm] * (scale[:, None] * 256.0)                # [NETW, 128]
        # blocked layout: [NBLK, 128, BLK], each block contiguous in DRAM
        # fp8 e4m3 with a 2^8 pre-scale (H ~1e-3 underflows e4m3 denormals);
        # the 2^-8 is folded into the PSUM->SBUF cast on device.
        ht = np.ascontiguousarray(
            np.clip(Hp.T, -448, 448).astype(FP8_NP)
            .reshape(CTXD, NBLK, BLK).transpose(1, 0, 2))

        # H-bias, stationary layout [128 ctx, 768]: col = chunk*128+m
        hbv = H[e][brows].astype(np.float32)                      # [768, 128]
        hb = np.ascontiguousarray((hbv * be).T).astype(BF16_NP)   # b1..b3 * beta

        # base biases, partition-major [128, 6]
        bbase = np.zeros((128, NBIAS), dtype=np.float32)
        bbase[:, 0] = b[0][e][0:128] * be
        bbase[:, 1] = b[0][e][128:256] * be
        bbase[:, 2] = b[1][e][0:128] * be
        bbase[:, 3] = b[1][e][128:256] * be
        bbase[:, 4] = b[2][e][0:128] * be
        bbase[:, 5] = b[2][e][128:256] * be

        w1t = np.ascontiguousarray(W[0][e].T).astype(BF16_NP)     # [64, 256]
        w2t = np.ascontiguousarray(
            (W[1][e].T / be).reshape(2, 128, WIDTH).transpose(1, 0, 2)
            .reshape(128, 2 * WIDTH)).astype(BF16_NP)             # [128, 512]
        w3t = np.ascontiguousarray(
            (W[2][e].T / be).reshape(2, 128, WIDTH).transpose(1, 0, 2)
            .reshape(128, 2 * WIDTH)).astype(BF16_NP)
        w4t = np.ascontiguousarray(
            (W[3][e].T / be).reshape(2, 128, DATA).transpose(1, 0, 2)
            .reshape(128, 2 * DATA)).astype(BF16_NP)              # [128, 128]

        gpe = np.zeros((128, ENVS // 2), dtype=np.float32)
        for p in range(ENVS // 2):
            gpe[0:DATA, p] = gate[2 * p, e]
            gpe[DATA:128, p] = gate[2 * p + 1, e]

        in_maps.append({
            "ht": ht, "hb": hb, "bbase": bbase, "ctxt": ctxT, "yt": yT,
            "w1t": w1t, "w2t": w2t, "w3t": w3t, "w4t": w4t,
            "gatep": np.ascontiguousarray(gpe),                   # [128, 8]
            "beta": np.array([be], dtype=np.float32),
        })
    return in_maps, host_term


# --------------------------------------------------------------------------
# device kernel (SPMD program, one expert per core)
# --------------------------------------------------------------------------
def _build_nc():
    if "nc" in _NC_CACHE:
        return _NC_CACHE["nc"]
    nc = bacc.Bacc()
    P = 128
    NPAIR = ENVS // 2

    ht = nc.declare_dram_parameter("ht", [NBLK, CTXD, BLK], FP8, isOutput=False)
    hb = nc.declare_dram_parameter("hb", [CTXD, NBIAS * 128], BF16, isOutput=False)
    bbase = nc.declare_dram_parameter("bbase", [P, NBIAS], F32, isOutput=False)
    ctxt = nc.declare_dram_parameter("ctxt", [CTXD, 32], BF16, isOutput=False)
    yt = nc.declare_dram_parameter("yt", [ENVS, DATA, NPTS], BF16, isOutput=False)
    w1t = nc.declare_dram_parameter("w1t", [DATA, WIDTH], BF16, isOutput=False)
    w2t = nc.declare_dram_parameter("w2t", [P, 2 * WIDTH], BF16, isOutput=False)
    w3t = nc.declare_dram_parameter("w3t", [P, 2 * WIDTH], BF16, isOutput=False)
    w4t = nc.declare_dram_parameter("w4t", [P, 2 * DATA], BF16, isOutput=False)
    gatep = nc.declare_dram_parameter("gatep", [P, NPAIR], F32, isOutput=False)
    beta = nc.declare_dram_parameter("beta", [1], F32, isOutput=False)
    out = nc.declare_dram_parameter("out", [ENVS, DATA, NPTS], BF16,
                                    isOutput=True)

    SILU = mybir.ActivationFunctionType.Silu

    with tile.TileContext(nc) as tc:
        with tc.tile_pool(name="dram", bufs=1, space="DRAM") as dram_pool, \
             tc.tile_pool(name="const", bufs=1) as const, \
             tc.tile_pool(name="psp", bufs=2, space="PSUM") as psp, \
             tc.tile_pool(name="dwall", bufs=1) as dwall, \
             tc.tile_pool(name="htp", bufs=2) as htp, \
             tc.tile_pool(name="cpp", bufs=2) as cpp, \
             tc.tile_pool(name="fw", bufs=2) as fwp, \
             tc.tile_pool(name="ypool", bufs=3) as ypool, \
             tc.tile_pool(name="hpool", bufs=22) as hpool, \
             tc.tile_pool(name="opool", bufs=2) as opool:
            # bf16 round-trip delta buffer: [blk, env, col]
            deltaT = dram_pool.tile([NBLK, ENVS, BLK], BF16)

            # constants loaded once
            ctx_sb = const.tile([CTXD, 32], BF16)
            nc.sync.dma_start(out=ctx_sb, in_=ctxt[:, :])
            beta_sb = const.tile([P, 1], F32)
            bap = beta[:]
            nc.sync.dma_start(
                out=beta_sb,
                in_=bass.AP(tensor=bap.tensor, offset=bap.offset,
                            ap=[[0, P]] + list(bap.ap)))
            gate_sb = const.tile([P, NPAIR], F32)
            nc.sync.dma_start(out=gate_sb, in_=gatep[:, :])
            w1t_sb = const.tile([DATA, WIDTH], BF16)
            nc.sync.dma_start(out=w1t_sb, in_=w1t[:, :])
            w2t_sb = const.tile([P, 2 * WIDTH], BF16)
            nc.sync.dma_start(out=w2t_sb, in_=w2t[:, :])
            w3t_sb = const.tile([P, 2 * WIDTH], BF16)
            nc.sync.dma_start(out=w3t_sb, in_=w3t[:, :])
            w4t_sb = const.tile([P, 2 * DATA], BF16)
            nc.sync.dma_start(out=w4t_sb, in_=w4t[:, :])
            hb_sb = const.tile([CTXD, NBIAS * 128], BF16)
            nc.sync.dma_start(out=hb_sb, in_=hb[:, :])
            bbase_sb = const.tile([P, NBIAS], F32)
            nc.sync.dma_start(out=bbase_sb, in_=bbase[:, :])

            # force the Silu ACT_TABLE_LOAD at t~9us instead of before the
            # first real activation
            warm = const.tile([P, 1], F32)
            nc.scalar.activation(out=warm, in_=beta_sb, func=SILU)

            # preload the first two envs' y tiles so the first l1 units
            # aren't gated on a cold sync queue
            y_tiles = {}
            for env in range(2):
                ysb = ypool.tile([DATA, NPTS], BF16, tag="y", name=f"y_{env}")
                nc.sync.dma_start(out=ysb, in_=yt[env])
                y_tiles[env] = ysb

            # ---------------- bias deltas (stationary-H matmuls) -----------
            fbias = const.tile([P, NBIAS * ENVS], F32)
            psb = psp.tile([P, 4 * 512], F32, tag="ps", name="psb")
            for c in range(NBIAS):
                nc.tensor.matmul(
                    psb[:, c * 16:(c + 1) * 16],
                    lhsT=hb_sb[:, c * 128:(c + 1) * 128],
                    rhs=ctx_sb[:, 0:ENVS],
                    start=True, stop=True)
            for c in range(NBIAS):
                nc.vector.tensor_scalar_add(
                    out=fbias[:, c * 16:(c + 1) * 16],
                    in0=psb[:, c * 16:(c + 1) * 16],
                    scalar1=bbase_sb[:, c:c + 1])

            # all-env delta buffers; w2/w3/w4 split per contraction half (kk)
            # so every regather is a contiguous-partition tile-sliced DMA
            dw1 = dwall.tile([DATA, ENVS * WIDTH], BF16, name="dw1")
            dw2 = [dwall.tile([P, ENVS * WIDTH], BF16, name=f"dw2k{k}")
                   for k in range(2)]
            dw3 = [dwall.tile([P, ENVS * WIDTH], BF16, name=f"dw3k{k}")
                   for k in range(2)]
            dw4 = [dwall.tile([P, ENVS * DATA], BF16, name=f"dw4k{k}")
                   for k in range(2)]

            hs = {}      # env -> dict of live tiles

            # ---------------- delta-phase emission ----------------
            ht_tiles = {}

            def emit_load(jb):
                """Prefetch one 4-block H group (issued 2 groups ahead)."""
                htg = htp.tile([CTXD, 4 * BLK], FP8, tag="ht", name=f"ht_{jb}")
                nc.gpsimd.dma_start(
                    out=htg.rearrange("c (j b) -> c j b", j=4),
                    in_=ht[4 * jb:4 * jb + 4].rearrange("j c b -> c j b"))
                ht_tiles[jb] = htg

            def emit_group(jb):
                """Stream 4 H-blocks: 16 matmuls, 1 cast, 4 stores.

                PSUM layout: partition quadrant = block (PE column tile),
                cols = the block's 2048 delta columns. Each block's 16 valid
                env rows are then a contiguous [16, 2048] slab in SBUF, so
                the deltaT store is a plain 2-D tile-sliced DMA.
                """
                htg = ht_tiles.pop(jb)
                cp4 = cpp.tile([P, 4 * 512], BF16, tag="cp", name=f"cp4_{jb}")
                ps = psp.tile([P, 4 * 512], F32, tag="ps", name=f"psS{jb}")
                for j4 in range(4):
                    for g in range(4):
                        nc.tensor.matmul(
                            ps[32 * j4:32 * j4 + 32,
                               g * 512:(g + 1) * 512],
                            lhsT=ctx_sb,
                            rhs=htg[:, j4 * BLK + g * 512:
                                    j4 * BLK + (g + 1) * 512],
                            start=True, stop=True,
                            tile_position=(0, 32 * j4),
                        )
                nc.vector.tensor_scalar_mul(
                    out=cp4, in0=ps, scalar1=1.0 / 256.0)
                for j4 in range(4):
                    nc.gpsimd.dma_start(
                        out=deltaT[4 * jb + j4],
                        in_=cp4[32 * j4:32 * j4 + ENVS, :])

            def emit_regather_block(b):
                """Regather delta block b from deltaT into its dw tile.

                Every transfer: contiguous-partition dest slice, pure
                tile-sliced/rearranged APs on both sides (dep-tracked).
                """
                if b < 8:                       # W1: rows d*256+w, d in 8b..
                    nc.sync.dma_start(
                        out=dw1[8 * b:8 * b + 8, :]
                        .rearrange("p (e w) -> p e w", e=ENVS),
                        in_=deltaT[b].rearrange("e (p w) -> p e w", p=8))
                elif b < 40:                    # W2: rows w*256+v
                    jl = (b - 8) % 16
                    kk = (b - 8) // 16
                    nc.sync.dma_start(
                        out=dw2[kk][8 * jl:8 * jl + 8, :]
                        .rearrange("p (e v) -> p e v", e=ENVS),
                        in_=deltaT[b].rearrange("e (p v) -> p e v", p=8))
                elif b < 72:                    # W3: rows w*256+v
                    jl = (b - 40) % 16
                    kk = (b - 40) // 16
                    nc.sync.dma_start(
                        out=dw3[kk][8 * jl:8 * jl + 8, :]
                        .rearrange("p (e v) -> p e v", e=ENVS),
                        in_=deltaT[b].rearrange("e (p v) -> p e v", p=8))
                else:                           # W4: rows w*64+d
                    jl = (b - 72) % 4
                    kk = (b - 72) // 4
                    nc.sync.dma_start(
                        out=dw4[kk][32 * jl:32 * jl + 32, :]
                        .rearrange("p (e d) -> p e d", e=ENVS),
                        in_=deltaT[b].rearrange("e (p d) -> p e d", p=32))

            # ---------------- MLP stage units ----------------
            def emit_l1_half(env, mt):
                if mt == 0:
                    fw1 = fwp.tile([DATA, WIDTH], BF16, tag="fw1",
                                   name=f"fw1_{env}")
                    nc.vector.tensor_add(
                        out=fw1, in0=w1t_sb,
                        in1=dw1[:, env * WIDTH:(env + 1) * WIDTH])
                    if env in y_tiles:
                        ysb = y_tiles.pop(env)
                    else:
                        ysb = ypool.tile([DATA, NPTS], BF16, tag="y",
                                         name=f"y_{env}")
                        nc.sync.dma_start(out=ysb, in_=yt[env])
                    hs[env] = {"fw": fw1, "y": ysb, "h": [None, None]}
                fw1, ysb = hs[env]["fw"], hs[env]["y"]
                ps1 = psp.tile([P, NPTS], F32, tag="ps",
                               name=f"ps1_{env}_{mt}")
                for t in range(4):
                    nc.tensor.matmul(
                        ps1[:, t * 512:(t + 1) * 512],
                        lhsT=fw1[:, mt * P:(mt + 1) * P],
                        rhs=ysb[:, t * 512:(t + 1) * 512],
                        start=True, stop=True)
                ht1 = hpool.tile([P, NPTS], BF16, tag="h",
                                 name=f"h1_{env}_{mt}")
                nc.scalar.activation(
                    out=ht1, in_=ps1[:, :], func=SILU,
                    bias=fbias[:, mt * 16 + env:mt * 16 + env + 1],
                    scale=beta_sb[:, 0:1])
                hs[env]["h"][mt] = ht1

            def emit_l23_half(env, li, mm):
                if mm == 0:
                    fw_l = fwp.tile([P, 2 * WIDTH], BF16, tag=f"fw{2 + li}",
                                    name=f"fw{2 + li}_{env}")
                    wt_sb = w2t_sb if li == 0 else w3t_sb
                    dwl = dw2 if li == 0 else dw3
                    for kk in range(2):
                        nc.vector.tensor_add(
                            out=fw_l[:, kk * WIDTH:(kk + 1) * WIDTH],
                            in0=wt_sb[:, kk * WIDTH:(kk + 1) * WIDTH],
                            in1=dwl[kk][:, env * WIDTH:(env + 1) * WIDTH])
                    hs[env]["fw"] = fw_l
                    hs[env]["hn"] = [None, None]
                fw_l, hprev = hs[env]["fw"], hs[env]["h"]
                psl = psp.tile([P, NPTS], F32, tag="ps",
                               name=f"psl_{env}_{li}_{mm}")
                for kk in range(2):
                    for t in range(4):
                        nc.tensor.matmul(
                            psl[:, t * 512:(t + 1) * 512],
                            lhsT=fw_l[:, kk * WIDTH + mm * P:
                                      kk * WIDTH + (mm + 1) * P],
                            rhs=hprev[kk][:, t * 512:(t + 1) * 512],
                            start=(kk == 0), stop=(kk == 1))
                htl = hpool.tile([P, NPTS], BF16, tag="h",
                                 name=f"h{2 + li}_{env}_{mm}")
                nc.scalar.activation(
                    out=htl, in_=psl[:, :], func=SILU,
                    bias=fbias[:, (2 + 2 * li + mm) * 16 + env:
                               (2 + 2 * li + mm) * 16 + env + 1],
                    scale=beta_sb[:, 0:1])
                hs[env]["hn"][mm] = htl
                if mm == 1:
                    hs[env]["h"] = hs[env]["hn"]
                    hs[env]["hn"] = None

            def emit_l4_pair(p):
                """Two envs' L4 in separate PE column tiles, one PSUM tile."""
                envs2 = (2 * p, 2 * p + 1)
                ps4 = psp.tile([P, NPTS], F32, tag="ps", name=f"ps4_{p}")
                for half, env in enumerate(envs2):
                    fw4 = fwp.tile([P, 2 * DATA], BF16, tag=f"fw4{half}",
                                   name=f"fw4_{env}")
                    for kk in range(2):
                        nc.vector.tensor_add(
                            out=fw4[:, kk * DATA:(kk + 1) * DATA],
                            in0=w4t_sb[:, kk * DATA:(kk + 1) * DATA],
                            in1=dw4[kk][:, env * DATA:(env + 1) * DATA])
                    hprev = hs[env]["h"]
                    for kk in range(2):
                        for t in range(4):
                            nc.tensor.matmul(
                                ps4[half * DATA:(half + 1) * DATA,
                                    t * 512:(t + 1) * 512],
                                lhsT=fw4[:, kk * DATA:(kk + 1) * DATA],
                                rhs=hprev[kk][:, t * 512:(t + 1) * 512],
                                start=(kk == 0), stop=(kk == 1),
                                tile_position=(0, half * DATA))
                    hs.pop(env)
                osb = opool.tile([P, NPTS], BF16, tag="osb", name=f"osb_{p}")
                nc.vector.tensor_scalar_mul(
                    out=osb, in0=ps4[:, :], scalar1=gate_sb[:, p:p + 1])
                nc.sync.dma_start(
                    out=out[2 * p:2 * p + 2].rearrange("e d n -> (e d) n"),
                    in_=osb)

            # ---------------- lag-gated deepest-first schedule -------------
            # STAGES per env: 0..5 = l1a,l1b,l2a,l2b,l3a,l3b; 6 = awaiting l4.
            # A unit is emittable only >=LAG ACT-units after the unit that
            # produces its matmul input, so in-order engine queues pipeline
            # across envs and the ACT engine never waits on a same-env chain.
            #
            # h-pool FIFO gate: hpool slot N+bufs reuses slot N, so a unit
            # may allocate its k h-tiles only when the tiles (bufs-k) back in
            # allocation order are retired (their last reader is emitted).
            LAG = 2
            HBUFS = 22               # must match hpool bufs
            next_stage = {e: 0 for e in range(ENVS)}
            emitted_at = {}
            state = {"n": 0}
            unlocked = {"l1": False, "l2": False, "l3": False, "l4": False}
            l4_done = [False] * NPAIR
            h_retired = []           # per h-allocation: last reader emitted?
            h_idx = {}               # (env, layer) -> [alloc indices]

            state_h = {"unret": 0}

            def h_can_alloc(k, reserve):
                if state_h["unret"] + k > HBUFS - reserve:
                    return False
                for i in range(k):
                    back = len(h_retired) + i - HBUFS
                    if back >= 0 and not h_retired[back]:
                        return False
                return True

            def h_alloc(env, layer):
                h_idx.setdefault((env, layer), []).append(len(h_retired))
                h_retired.append(False)
                state_h["unret"] += 1

            def h_retire(env, layer):
                for i in h_idx.pop((env, layer), []):
                    h_retired[i] = True
                    state_h["unret"] -= 1

            def emit_stage(e):
                si = next_stage[e]
                if si == 0:
                    emit_l1_half(e, 0)
                elif si == 1:
                    emit_l1_half(e, 1)
                elif si == 2:
                    emit_l23_half(e, 0, 0)
                elif si == 3:
                    emit_l23_half(e, 0, 1)
                elif si == 4:
                    emit_l23_half(e, 1, 0)
                else:
                    emit_l23_half(e, 1, 1)
                h_alloc(e, si // 2)
                if si == 3:
                    h_retire(e, 0)          # l2b's MMs are h1's last readers
                elif si == 5:
                    h_retire(e, 1)          # l3b's MMs are h2's last readers
                emitted_at[(si, e)] = state["n"]
                state["n"] += 1
                next_stage[e] = si + 1

            def stage_ready(e, force=False):
                si = next_stage[e]
                if si >= 6:
                    return False
                # l1 pairs grow the live set (+2, retired only at l2b);
                # l2/l3 pairs are net zero but need 2 transient slots
                if si == 0:
                    ok = h_can_alloc(2, 4)
                elif si in (2, 4):
                    ok = h_can_alloc(2, 0)
                else:
                    ok = h_can_alloc(1, 0)
                if not ok:
                    return False
                if si <= 1:
                    return unlocked["l1"]
                if si <= 3:
                    if not unlocked["l2"]:
                        return False
                    if si == 2 and not force:
                        return state["n"] - emitted_at[(1, e)] >= LAG
                    return True
                if not unlocked["l3"]:
                    return False
                if si == 4 and not force:
                    return state["n"] - emitted_at[(3, e)] >= LAG
                return True

            def l4_ready(p, force=False):
                if not unlocked["l4"] or l4_done[p]:
                    return False
                a, b = 2 * p, 2 * p + 1
                if next_stage[a] < 6 or next_stage[b] < 6:
                    return False
                return force or state["n"] - emitted_at[(5, b)] >= LAG

            def try_emit_one():
                for p in range(NPAIR):
                    if l4_ready(p):
                        emit_l4_pair(p)
                        h_retire(2 * p, 2)
                        h_retire(2 * p + 1, 2)
                        l4_done[p] = True
                        return True
                best = None
                for e in range(ENVS):
                    if stage_ready(e):
                        if best is None or next_stage[e] > next_stage[best]:
                            best = e
                if best is None:
                    return False
                emit_stage(best)
                return True

            # ---------------- fused emission ----------------
            # each group's 4 regather DMAs are emitted right behind its
            # stores; stage unlocks follow the last relevant regather
            emit_load(0)
            emit_load(1)
            for jb in range(20):
                if jb + 2 < 20:
                    emit_load(jb + 2)
                emit_group(jb)
                for b in range(4 * jb, 4 * jb + 4):
                    emit_regather_block(b)
                if jb == 1:
                    unlocked["l1"] = True
                elif jb == 10:
                    unlocked["l2"] = True       # W2 done at jb=9
                elif jb == 18:
                    unlocked["l3"] = True       # W3 done at jb=17
                if jb >= 2:
                    for _ in range(3):
                        try_emit_one()
            unlocked["l4"] = True
            while (any(next_stage[e] < 6 for e in range(ENVS))
                   or not all(l4_done)):
                if not try_emit_one():
                    forced = False
                    for p in range(NPAIR):
                        if l4_ready(p, force=True):
                            emit_l4_pair(p)
                            h_retire(2 * p, 2)
                            h_retire(2 * p + 1, 2)
                            l4_done[p] = True
                            forced = True
                            break
                    if not forced:
                        for e in range(ENVS):
                            if stage_ready(e, force=True):
                                emit_stage(e)
                                forced = True
                                break
                    if not forced:
                        raise AssertionError("emission worklist stuck")

    nc.compile()
    _NC_CACHE["nc"] = nc
    return nc


# --------------------------------------------------------------------------
# entry point
# --------------------------------------------------------------------------
def kernel(t, y, ctx, W1, b1, W2, b2, W3, b3, W4, b4, H, G, beta):
    global LAST_RESULTS
    y = np.asarray(y, np.float32)
    ctx = np.asarray(ctx, np.float32)
    H = np.asarray(H, np.float32)
    G = np.asarray(G, np.float32)
    beta = np.asarray(beta, np.float32)
    W = [np.asarray(w, np.float32) for w in (W1, W2, W3, W4)]
    b = [np.asarray(x, np.float32) for x in (b1, b2, b3, b4)]

    in_maps, host_term = _prep_inputs(y, ctx, W, b, H, G, beta)
    nc = _build_nc()
    res = run_bass_kernel_spmd(
        nc, in_maps, list(range(N_CORES)),
        trace=TRACE, trace_cores=None)
    LAST_RESULTS = res

    total = np.zeros((ENVS, DATA, NPTS), np.float32)
    for e in range(N_CORES):
        total += res.results[e]["out"]
    total += host_term[:, :, None]
    return np.ascontiguousarray(total.transpose(0, 2, 1))


def measure_exec_ns(inputs, iters=64, warmup=4):
    """Steady-state per-execution time of the compiled NEFF on 8 cores.

    Used by test.py only; the grading path never calls this.
    """
    import time
    import jax
    from jax.sharding import Mesh, PartitionSpec, NamedSharding
    from jax.experimental.shard_map import shard_map
    from concourse import bass2jax, mybir as _mybir

    y = np.asarray(inputs["y"], np.float32)
    ctx = np.asarray(inputs["ctx"], np.float32)
    H = np.asarray(inputs["H"], np.float32)
    G = np.asarray(inputs["G"], np.float32)
    beta = np.asarray(inputs["beta"], np.float32)
    W = [np.asarray(inputs[k], np.float32) for k in ("W1", "W2", "W3", "W4")]
    b = [np.asarray(inputs[k], np.float32) for k in ("b1", "b2", "b3", "b4")]
    in_maps, _ = _prep_inputs(y, ctx, W, b, H, G, beta)
    nc = _build_nc()

    bass2jax.install_neuronx_cc_hook()
    partition_name = nc.partition_id_tensor.name if nc.partition_id_tensor else None
    in_names, out_names, out_avals, zero_outs = [], [], [], []
    for alloc in nc.m.functions[0].allocations:
        if not isinstance(alloc, _mybir.MemoryLocationSet):
            continue
        name = alloc.memorylocations[0].name
        if alloc.kind == "ExternalInput":
            if name != partition_name:
                in_names.append(name)
        elif alloc.kind == "ExternalOutput":
            shape = tuple(alloc.tensor_shape)
            dtype = _mybir.dt.np(alloc.dtype)
            out_names.append(name)
            out_avals.append(jax.core.ShapedArray(shape, dtype))
            zero_outs.append(np.zeros(shape, dtype))
    n_params = len(in_names)
    all_in_names = in_names + out_names
    if partition_name is not None:
        all_in_names.append(partition_name)

    def _body(*args):
        operands = list(args)
        if partition_name is not None:
            operands.append(bass2jax.partition_id_tensor())
        outs = bass2jax._bass_exec_p.bind(
            *operands,
            out_avals=tuple(out_avals),
            in_names=tuple(all_in_names),
            out_names=tuple(out_names),
            lowering_input_output_aliases=(),
            sim_require_finite=True,
            sim_require_nnan=True,
            nc=nc,
        )
        return tuple(outs)

    devices = jax.devices()[:N_CORES]
    mesh = Mesh(np.asarray(devices), ("core",))
    nspec = NamedSharding(mesh, PartitionSpec("core"))
    n_all = n_params + len(out_names)
    sharded = jax.jit(
        shard_map(_body, mesh=mesh,
                  in_specs=(PartitionSpec("core"),) * n_all,
                  out_specs=(PartitionSpec("core"),) * len(out_names),
                  check_rep=False),
        keep_unused=True)

    concat_in = [
        np.concatenate([np.asarray(in_maps[c][k]) for c in range(N_CORES)], axis=0)
        for k in in_names
    ] + [np.zeros((N_CORES * z.shape[0], *z.shape[1:]), z.dtype) for z in zero_outs]
    dev_in = [jax.device_put(a, nspec) for a in concat_in]

    for _ in range(warmup):
        outs = sharded(*dev_in)
    jax.block_until_ready(outs)

    t0 = time.perf_counter()
    for _ in range(iters):
        outs = sharded(*dev_in)
    jax.block_until_ready(outs)
    t1 = time.perf_counter()
    per_call = (t1 - t0) / iters

    return {"pipelined_ns": per_call * 1e9}


if __name__ == "__main__":
    _build_nc()
    print("IR build OK")


# revision 41
# speedup vs baseline: 1.0477x; 1.0477x over previous
"""MixER MoE-hypernetwork kernel for 8 Trainium2 NeuronCores.

Expert-parallel: core e handles expert e (NEXP == n_cores == 8).

v2 schedule: the Scalar/ACT engine is the hard bottleneck (96 Silu
ACTIVATEs x ~2.05us = ~197us/core minimum), so everything else is arranged
to hide under a continuously-fed ACT queue:
  - H-block loads + deltaT stores grouped (1 DMA per 4-block group) and
    issued from the GpSimd queue; regathers/y/out on Sync. The Scalar queue
    carries ACTIVATEs only.
  - MLP emitted via a lag-gated deepest-first worklist: a stage unit is
    only emitted >=2 ACT-units after the unit producing its input, so the
    in-order engine queues pipeline across envs with no ACT stalls.
  - L4 paired: two envs' final matmuls run concurrently in separate PE
    column tiles into one [128,2048] PSUM tile; single full-width DVE
    gate-multiply epilogue; b4-bias gate term hoisted to the host.
  - per-block PSUM casts in the delta phase free PSUM banks early.
Host: gate softmax, y transpose, H permute/scale/cast to fp8, sum of the 8
per-expert partials + gate-weighted fb4 bias term.
"""
import os
from contextlib import ExitStack

import numpy as np
import ml_dtypes

import concourse.bass as bass
import concourse.bacc as bacc
import concourse.tile as tile
from concourse import mybir
from concourse.bass_utils import run_bass_kernel_spmd

# ---- problem dims (hardcoded; must match the grader's setup_inputs) ----
DATA, WIDTH, CTXD, NEXP, ENVS, NPTS = 64, 256, 128, 8, 16, 2048
SIZES = [WIDTH * DATA, WIDTH, WIDTH * WIDTH, WIDTH, WIDTH * WIDTH, WIDTH,
         DATA * WIDTH, DATA]
OFFS = np.cumsum([0] + SIZES)
BLK = 2048
NBLK = 80                          # weight regions only: 163840 = 80*2048
NETW = NBLK * BLK

# device-layout offsets into the weight-delta stream: [W1T | W2T | W3T | W4T]
O_W1, O_W2, O_W3, O_W4 = 0, 16384, 81920, 147456
NBIAS = 6                          # bias chunks of 128 rows (b1,b1,b2,b2,b3,b3)

F32 = mybir.dt.float32
BF16 = mybir.dt.bfloat16
BF16_NP = ml_dtypes.bfloat16
FP8 = mybir.dt.float8e4
FP8_NP = ml_dtypes.float8_e4m3

N_CORES = 8
TRACE = os.environ.get("MIXER_TRACE", "0") == "1"

if TRACE:
    # The agent image's antenv lacks axon_hooks, so run_bass_kernel_spmd's
    # trace path can't find the NTFF profile hook. Shim it with the ctypes
    # hook factory that trn_boot ships. Profiling-only; inert when TRACE=0.
    try:
        from antenv.axon_hooks import get_axon_ntff_profile_hook  # noqa: F401
    except ImportError:
        import sys as _sys
        import types as _types
        try:
            from trn_agent_boot.trn_boot import _ntff_profile_via_ctypes
            _hook = _ntff_profile_via_ctypes("/opt/axon/libaxon_pjrt.so")
            import antenv as _antenv
            _mod = _types.ModuleType("antenv.axon_hooks")
            _mod.get_axon_ntff_profile_hook = lambda: _hook
            _mod.set_axon_ntff_profile_hook = lambda h: None
            _sys.modules["antenv.axon_hooks"] = _mod
            _antenv.axon_hooks = _mod
        except Exception as _e:  # pragma: no cover - profiling is best-effort
            print(f"NTFF hook shim failed: {_e}")

LAST_RESULTS = None  # BassKernelResults of the most recent run (for test.py)

_NC_CACHE = {}
_PERM_CACHE = {}


# --------------------------------------------------------------------------
# host-side preprocessing
# --------------------------------------------------------------------------
def _build_perm():
    """perm[new_row] = old_row of H's NET axis, weight regions only."""
    if "perm" in _PERM_CACHE:
        return _PERM_CACHE["perm"]
    perm = np.zeros(NETW, dtype=np.int64)
    # W1: orig OFFS[0] + w*DATA+d  -> new O_W1 + d*WIDTH+w   ([64,256] = fW1T)
    d, w = np.meshgrid(np.arange(DATA), np.arange(WIDTH), indexing="ij")
    perm[O_W1 + (d * WIDTH + w).ravel()] = OFFS[0] + (w * DATA + d).ravel()
    # W2/W3: orig + v*WIDTH+w (v,w) -> new + w*WIDTH+v  ([256,256] = fW2T)
    w2, v2 = np.meshgrid(np.arange(WIDTH), np.arange(WIDTH), indexing="ij")
    perm[O_W2 + (w2 * WIDTH + v2).ravel()] = OFFS[2] + (v2 * WIDTH + w2).ravel()
    perm[O_W3 + (w2 * WIDTH + v2).ravel()] = OFFS[4] + (v2 * WIDTH + w2).ravel()
    # W4: orig + d*WIDTH+w (d,w) -> new + w*DATA+d  ([256,64] = fW4T)
    d4, w4 = np.meshgrid(np.arange(DATA), np.arange(WIDTH), indexing="ij")
    perm[O_W4 + (w4 * DATA + d4).ravel()] = OFFS[6] + (d4 * WIDTH + w4).ravel()
    _PERM_CACHE["perm"] = perm
    return perm


def _build_scale(beta_e):
    ib = np.float32(1.0 / beta_e)
    scale = np.ones(NETW, dtype=np.float32)
    scale[O_W2:O_W2 + WIDTH * WIDTH] = ib
    scale[O_W3:O_W3 + WIDTH * WIDTH] = ib
    scale[O_W4:O_W4 + WIDTH * DATA] = ib
    return scale


def _bias_rows():
    """orig H rows for the 6 bias chunks of 128 (b1,b1,b2,b2,b3,b3)."""
    rows = np.zeros(NBIAS * 128, dtype=np.int64)
    rows[0:256] = OFFS[1] + np.arange(WIDTH)          # b1
    rows[256:512] = OFFS[3] + np.arange(WIDTH)        # b2
    rows[512:768] = OFFS[5] + np.arange(WIDTH)        # b3
    return rows


def _prep_inputs(y, ctx, W, b, H, G, beta):
    """Returns (in_maps, host_term): one dict per core + [ENVS, DATA] bias
    term (gate-weighted fb4) to add on the host after summing cores."""
    perm = _build_perm()
    brows = _bias_rows()

    # gate softmax on host (tiny)
    logits = ctx.astype(np.float32) @ G.astype(np.float32).T      # [B, E]
    m = logits.max(-1, keepdims=True)
    eg = np.exp(logits - m)
    gate = (eg / eg.sum(-1, keepdims=True)).astype(np.float32)

    # gate-weighted fb4 term: sum_e gate[b,e] * (b4[e] + H4[e] @ ctx[b])
    fb4 = np.stack([
        b[3][e][None, :] + ctx.astype(np.float32)
        @ H[e][OFFS[7]:OFFS[7] + DATA].astype(np.float32).T
        for e in range(NEXP)], axis=1)                            # [B, E, 64]
    host_term = np.einsum("be,bed->bd", gate, fb4).astype(np.float32)

    yT = np.ascontiguousarray(y.transpose(0, 2, 1)).astype(BF16_NP)
    # ctx^T padded to 32 cols so delta matmuls write full PSUM quadrants
    ctxT = np.zeros((CTXD, 32), dtype=BF16_NP)
    ctxT[:, :ENVS] = ctx.T.astype(BF16_NP)

    in_maps = []
    for e in range(NEXP):
        be = float(beta[e])
        scale = _build_scale(be)
        Hp = H[e][perm] * (scale[:, None] * 256.0)                # [NETW, 128]
        # blocked layout: [NBLK, 128, BLK], each block contiguous in DRAM
        # fp8 e4m3 with a 2^8 pre-scale (H ~1e-3 underflows e4m3 denormals);
        # the 2^-8 is folded into the PSUM->SBUF cast on device.
        ht = np.ascontiguousarray(
            np.clip(Hp.T, -448, 448).astype(FP8_NP)
            .reshape(CTXD, NBLK, BLK).transpose(1, 0, 2))

        # H-bias, stationary layout [128 ctx, 768]: col = chunk*128+m
        hbv = H[e][brows].astype(np.float32)                      # [768, 128]
        hb = np.ascontiguousarray((hbv * be).T).astype(BF16_NP)   # b1..b3 * beta

        # base biases, partition-major [128, 6]
        bbase = np.zeros((128, NBIAS), dtype=np.float32)
        bbase[:, 0] = b[0][e][0:128] * be
        bbase[:, 1] = b[0][e][128:256] * be
        bbase[:, 2] = b[1][e][0:128] * be
        bbase[:, 3] = b[1][e][128:256] * be
        bbase[:, 4] = b[2][e][0:128] * be
        bbase[:, 5] = b[2][e][128:256] * be

        w1t = np.ascontiguousarray(W[0][e].T).astype(BF16_NP)     # [64, 256]
        w2t = np.ascontiguousarray(
            (W[1][e].T / be).reshape(2, 128, WIDTH).transpose(1, 0, 2)
            .reshape(128, 2 * WIDTH)).astype(BF16_NP)             # [128, 512]
        w3t = np.ascontiguousarray(
            (W[2][e].T / be).reshape(2, 128, WIDTH).transpose(1, 0, 2)
            .reshape(128, 2 * WIDTH)).astype(BF16_NP)
        w4t = np.ascontiguousarray(
            (W[3][e].T / be).reshape(2, 128, DATA).transpose(1, 0, 2)
            .reshape(128, 2 * DATA)).astype(BF16_NP)              # [128, 128]

        gpe = np.zeros((128, ENVS // 2), dtype=np.float32)
        for p in range(ENVS // 2):
            gpe[0:DATA, p] = gate[2 * p, e]
            gpe[DATA:128, p] = gate[2 * p + 1, e]

        in_maps.append({
            "ht": ht, "hb": hb, "bbase": bbase, "ctxt": ctxT, "yt": yT,
            "w1t": w1t, "w2t": w2t, "w3t": w3t, "w4t": w4t,
            "gatep": np.ascontiguousarray(gpe),                   # [128, 8]
            "beta": np.array([be], dtype=np.float32),
        })
    return in_maps, host_term


# --------------------------------------------------------------------------
# device kernel (SPMD program, one expert per core)
# --------------------------------------------------------------------------
def _build_nc():
    if "nc" in _NC_CACHE:
        return _NC_CACHE["nc"]
    nc = bacc.Bacc()
    P = 128
    NPAIR = ENVS // 2

    ht = nc.declare_dram_parameter("ht", [NBLK, CTXD, BLK], FP8, isOutput=False)
    hb = nc.declare_dram_parameter("hb", [CTXD, NBIAS * 128], BF16, isOutput=False)
    bbase = nc.declare_dram_parameter("bbase", [P, NBIAS], F32, isOutput=False)
    ctxt = nc.declare_dram_parameter("ctxt", [CTXD, 32], BF16, isOutput=False)
    yt = nc.declare_dram_parameter("yt", [ENVS, DATA, NPTS], BF16, isOutput=False)
    w1t = nc.declare_dram_parameter("w1t", [DATA, WIDTH], BF16, isOutput=False)
    w2t = nc.declare_dram_parameter("w2t", [P, 2 * WIDTH], BF16, isOutput=False)
    w3t = nc.declare_dram_parameter("w3t", [P, 2 * WIDTH], BF16, isOutput=False)
    w4t = nc.declare_dram_parameter("w4t", [P, 2 * DATA], BF16, isOutput=False)
    gatep = nc.declare_dram_parameter("gatep", [P, NPAIR], F32, isOutput=False)
    beta = nc.declare_dram_parameter("beta", [1], F32, isOutput=False)
    out = nc.declare_dram_parameter("out", [ENVS, DATA, NPTS], BF16,
                                    isOutput=True)

    SILU = mybir.ActivationFunctionType.Silu

    with tile.TileContext(nc) as tc:
        with tc.tile_pool(name="dram", bufs=1, space="DRAM") as dram_pool, \
             tc.tile_pool(name="const", bufs=1) as const, \
             tc.tile_pool(name="dwall", bufs=1) as dwall, \
             tc.tile_pool(name="htp", bufs=2) as htp, \
             tc.tile_pool(name="cpp", bufs=2) as cpp, \
             tc.tile_pool(name="fw", bufs=2) as fwp, \
             tc.tile_pool(name="ypool", bufs=3) as ypool, \
             tc.tile_pool(name="hpool", bufs=22) as hpool, \
             tc.tile_pool(name="opool", bufs=2) as opool:
            # bf16 round-trip delta buffer: [blk, env, col]
            deltaT = dram_pool.tile([NBLK, ENVS, BLK], BF16)

            # PSUM phase 1: delta pool (bf16 tiles, 2 banks each — exact:
            # the 2^-8 scale is a power of two and deltaT is bf16 anyway)
            # runs at DMA pace, fully decoupled from the MLP drain ring,
            # which uses 2-bank [128,1024] f32 tiles meanwhile. Phase 2
            # (delta PSUM released): full-width [128,2048] MLP ring.
            ph1 = ExitStack()
            psd = ph1.enter_context(
                tc.tile_pool(name="psd", bufs=2, space="PSUM"))
            psm1 = ph1.enter_context(
                tc.tile_pool(name="psm1", bufs=2, space="PSUM"))
            cur = {"pool": psm1, "fd": 1024}

            # constants loaded once
            ctx_sb = const.tile([CTXD, 32], BF16)
            nc.sync.dma_start(out=ctx_sb, in_=ctxt[:, :])
            beta_sb = const.tile([P, 1], F32)
            bap = beta[:]
            nc.sync.dma_start(
                out=beta_sb,
                in_=bass.AP(tensor=bap.tensor, offset=bap.offset,
                            ap=[[0, P]] + list(bap.ap)))
            gate_sb = const.tile([P, NPAIR], F32)
            nc.sync.dma_start(out=gate_sb, in_=gatep[:, :])
            w1t_sb = const.tile([DATA, WIDTH], BF16)
            nc.sync.dma_start(out=w1t_sb, in_=w1t[:, :])
            w2t_sb = const.tile([P, 2 * WIDTH], BF16)
            nc.sync.dma_start(out=w2t_sb, in_=w2t[:, :])
            w3t_sb = const.tile([P, 2 * WIDTH], BF16)
            nc.sync.dma_start(out=w3t_sb, in_=w3t[:, :])
            w4t_sb = const.tile([P, 2 * DATA], BF16)
            nc.sync.dma_start(out=w4t_sb, in_=w4t[:, :])
            hb_sb = const.tile([CTXD, NBIAS * 128], BF16)
            nc.sync.dma_start(out=hb_sb, in_=hb[:, :])
            bbase_sb = const.tile([P, NBIAS], F32)
            nc.sync.dma_start(out=bbase_sb, in_=bbase[:, :])

            # force the Silu ACT_TABLE_LOAD at t~9us instead of before the
            # first real activation
            warm = const.tile([P, 1], F32)
            nc.scalar.activation(out=warm, in_=beta_sb, func=SILU)

            # preload the first two envs' y tiles so the first l1 units
            # aren't gated on a cold sync queue
            y_tiles = {}
            for env in range(2):
                ysb = ypool.tile([DATA, NPTS], BF16, tag="y", name=f"y_{env}")
                nc.sync.dma_start(out=ysb, in_=yt[env])
                y_tiles[env] = ysb

            # ---------------- bias deltas (stationary-H matmuls) -----------
            fbias = const.tile([P, NBIAS * ENVS], F32)
            psb = psm1.tile([P, 1024], F32, tag="ps", name="psb")
            for c in range(NBIAS):
                nc.tensor.matmul(
                    psb[:, c * 16:(c + 1) * 16],
                    lhsT=hb_sb[:, c * 128:(c + 1) * 128],
                    rhs=ctx_sb[:, 0:ENVS],
                    start=True, stop=True)
            for c in range(NBIAS):
                nc.vector.tensor_scalar_add(
                    out=fbias[:, c * 16:(c + 1) * 16],
                    in0=psb[:, c * 16:(c + 1) * 16],
                    scalar1=bbase_sb[:, c:c + 1])

            # all-env delta buffers; w2/w3/w4 split per contraction half (kk)
            # so every regather is a contiguous-partition tile-sliced DMA
            dw1 = dwall.tile([DATA, ENVS * WIDTH], BF16, name="dw1")
            dw2 = [dwall.tile([P, ENVS * WIDTH], BF16, name=f"dw2k{k}")
                   for k in range(2)]
            dw3 = [dwall.tile([P, ENVS * WIDTH], BF16, name=f"dw3k{k}")
                   for k in range(2)]
            dw4 = [dwall.tile([P, ENVS * DATA], BF16, name=f"dw4k{k}")
                   for k in range(2)]

            hs = {}      # env -> dict of live tiles

            # ---------------- delta-phase emission ----------------
            ht_tiles = {}

            def emit_load(jb):
                """Prefetch one 4-block H group (issued 2 groups ahead)."""
                htg = htp.tile([CTXD, 4 * BLK], FP8, tag="ht", name=f"ht_{jb}")
                nc.gpsimd.dma_start(
                    out=htg.rearrange("c (j b) -> c j b", j=4),
                    in_=ht[4 * jb:4 * jb + 4].rearrange("j c b -> c j b"))
                ht_tiles[jb] = htg

            def emit_group(jb):
                """Stream 4 H-blocks: 16 matmuls, 1 cast, 4 stores.

                PSUM layout: partition quadrant = block (PE column tile),
                cols = the block's 2048 delta columns. Each block's 16 valid
                env rows are then a contiguous [16, 2048] slab in SBUF, so
                the deltaT store is a plain 2-D tile-sliced DMA.
                """
                htg = ht_tiles.pop(jb)
                cp4 = cpp.tile([P, 4 * 512], BF16, tag="cp", name=f"cp4_{jb}")
                for ch in range(2):             # column half of each block
                    ps = psd.tile([P, 1024], F32, tag="dps",
                                  name=f"psS{jb}_{ch}")
                    for j4 in range(4):
                        for g2 in range(2):
                            g = 2 * ch + g2
                            nc.tensor.matmul(
                                ps[32 * j4:32 * j4 + 32,
                                   g2 * 512:(g2 + 1) * 512],
                                lhsT=ctx_sb,
                                rhs=htg[:, j4 * BLK + g * 512:
                                        j4 * BLK + (g + 1) * 512],
                                start=True, stop=True,
                                tile_position=(0, 32 * j4),
                            )
                    nc.vector.tensor_scalar_mul(
                        out=cp4[:, ch * 1024:(ch + 1) * 1024],
                        in0=ps, scalar1=1.0 / 256.0)
                for j4 in range(4):
                    nc.gpsimd.dma_start(
                        out=deltaT[4 * jb + j4],
                        in_=cp4[32 * j4:32 * j4 + ENVS, :])

            def emit_regather_block(b):
                """Regather delta block b from deltaT into its dw tile.

                Every transfer: contiguous-partition dest slice, pure
                tile-sliced/rearranged APs on both sides (dep-tracked).
                """
                if b < 8:                       # W1: rows d*256+w, d in 8b..
                    nc.sync.dma_start(
                        out=dw1[8 * b:8 * b + 8, :]
                        .rearrange("p (e w) -> p e w", e=ENVS),
                        in_=deltaT[b].rearrange("e (p w) -> p e w", p=8))
                elif b < 40:                    # W2: rows w*256+v
                    jl = (b - 8) % 16
                    kk = (b - 8) // 16
                    nc.sync.dma_start(
                        out=dw2[kk][8 * jl:8 * jl + 8, :]
                        .rearrange("p (e v) -> p e v", e=ENVS),
                        in_=deltaT[b].rearrange("e (p v) -> p e v", p=8))
                elif b < 72:                    # W3: rows w*256+v
                    jl = (b - 40) % 16
                    kk = (b - 40) // 16
                    nc.sync.dma_start(
                        out=dw3[kk][8 * jl:8 * jl + 8, :]
                        .rearrange("p (e v) -> p e v", e=ENVS),
                        in_=deltaT[b].rearrange("e (p v) -> p e v", p=8))
                else:                           # W4: rows w*64+d
                    jl = (b - 72) % 4
                    kk = (b - 72) // 4
                    nc.sync.dma_start(
                        out=dw4[kk][32 * jl:32 * jl + 32, :]
                        .rearrange("p (e d) -> p e d", e=ENVS),
                        in_=deltaT[b].rearrange("e (p d) -> p e d", p=32))

            # ---------------- MLP stage units ----------------
            def emit_l1_half(env, mt):
                if mt == 0:
                    fw1 = fwp.tile([DATA, WIDTH], BF16, tag="fw1",
                                   name=f"fw1_{env}")
                    nc.vector.tensor_add(
                        out=fw1, in0=w1t_sb,
                        in1=dw1[:, env * WIDTH:(env + 1) * WIDTH])
                    if env in y_tiles:
                        ysb = y_tiles.pop(env)
                    else:
                        ysb = ypool.tile([DATA, NPTS], BF16, tag="y",
                                         name=f"y_{env}")
                        nc.sync.dma_start(out=ysb, in_=yt[env])
                    hs[env] = {"fw": fw1, "y": ysb, "h": [None, None]}
                fw1, ysb = hs[env]["fw"], hs[env]["y"]
                ht1 = hpool.tile([P, NPTS], BF16, tag="h",
                                 name=f"h1_{env}_{mt}")
                fd = cur["fd"]
                for dt_ in range(NPTS // fd):
                    ps1 = cur["pool"].tile([P, fd], F32, tag="ps",
                                           name=f"ps1_{env}_{mt}_{dt_}")
                    for t in range(fd // 512):
                        col = dt_ * fd + t * 512
                        nc.tensor.matmul(
                            ps1[:, t * 512:(t + 1) * 512],
                            lhsT=fw1[:, mt * P:(mt + 1) * P],
                            rhs=ysb[:, col:col + 512],
                            start=True, stop=True)
                    nc.scalar.activation(
                        out=ht1[:, dt_ * fd:(dt_ + 1) * fd], in_=ps1[:, :],
                        func=SILU,
                        bias=fbias[:, mt * 16 + env:mt * 16 + env + 1],
                        scale=beta_sb[:, 0:1])
                hs[env]["h"][mt] = ht1

            def emit_l23_half(env, li, mm):
                if mm == 0:
                    fw_l = fwp.tile([P, 2 * WIDTH], BF16, tag=f"fw{2 + li}",
                                    name=f"fw{2 + li}_{env}")
                    wt_sb = w2t_sb if li == 0 else w3t_sb
                    dwl = dw2 if li == 0 else dw3
                    for kk in range(2):
                        nc.vector.tensor_add(
                            out=fw_l[:, kk * WIDTH:(kk + 1) * WIDTH],
                            in0=wt_sb[:, kk * WIDTH:(kk + 1) * WIDTH],
                            in1=dwl[kk][:, env * WIDTH:(env + 1) * WIDTH])
                    hs[env]["fw"] = fw_l
                    hs[env]["hn"] = [None, None]
                fw_l, hprev = hs[env]["fw"], hs[env]["h"]
                htl = hpool.tile([P, NPTS], BF16, tag="h",
                                 name=f"h{2 + li}_{env}_{mm}")
                fd = cur["fd"]
                for dt_ in range(NPTS // fd):
                    psl = cur["pool"].tile([P, fd], F32, tag="ps",
                                           name=f"psl_{env}_{li}_{mm}_{dt_}")
                    for kk in range(2):
                        for t in range(fd // 512):
                            col = dt_ * fd + t * 512
                            nc.tensor.matmul(
                                psl[:, t * 512:(t + 1) * 512],
                                lhsT=fw_l[:, kk * WIDTH + mm * P:
                                          kk * WIDTH + (mm + 1) * P],
                                rhs=hprev[kk][:, col:col + 512],
                                start=(kk == 0), stop=(kk == 1))
                    nc.scalar.activation(
                        out=htl[:, dt_ * fd:(dt_ + 1) * fd], in_=psl[:, :],
                        func=SILU,
                        bias=fbias[:, (2 + 2 * li + mm) * 16 + env:
                                   (2 + 2 * li + mm) * 16 + env + 1],
                        scale=beta_sb[:, 0:1])
                hs[env]["hn"][mm] = htl
                if mm == 1:
                    hs[env]["h"] = hs[env]["hn"]
                    hs[env]["hn"] = None

            def emit_l4_pair(p):
                """Two envs' L4 in separate PE column tiles, one PSUM tile."""
                envs2 = (2 * p, 2 * p + 1)
                ps4 = cur["pool"].tile([P, NPTS], F32, tag="ps",
                                       name=f"ps4_{p}")
                for half, env in enumerate(envs2):
                    fw4 = fwp.tile([P, 2 * DATA], BF16, tag=f"fw4{half}",
                                   name=f"fw4_{env}")
                    for kk in range(2):
                        nc.vector.tensor_add(
                            out=fw4[:, kk * DATA:(kk + 1) * DATA],
                            in0=w4t_sb[:, kk * DATA:(kk + 1) * DATA],
                            in1=dw4[kk][:, env * DATA:(env + 1) * DATA])
                    hprev = hs[env]["h"]
                    for kk in range(2):
                        for t in range(4):
                            nc.tensor.matmul(
                                ps4[half * DATA:(half + 1) * DATA,
                                    t * 512:(t + 1) * 512],
                                lhsT=fw4[:, kk * DATA:(kk + 1) * DATA],
                                rhs=hprev[kk][:, t * 512:(t + 1) * 512],
                                start=(kk == 0), stop=(kk == 1),
                                tile_position=(0, half * DATA))
                    hs.pop(env)
                osb = opool.tile([P, NPTS], BF16, tag="osb", name=f"osb_{p}")
                nc.vector.tensor_scalar_mul(
                    out=osb, in0=ps4[:, :], scalar1=gate_sb[:, p:p + 1])
                nc.sync.dma_start(
                    out=out[2 * p:2 * p + 2].rearrange("e d n -> (e d) n"),
                    in_=osb)

            # ---------------- lag-gated deepest-first schedule -------------
            # STAGES per env: 0..5 = l1a,l1b,l2a,l2b,l3a,l3b; 6 = awaiting l4.
            # A unit is emittable only >=LAG ACT-units after the unit that
            # produces its matmul input, so in-order engine queues pipeline
            # across envs and the ACT engine never waits on a same-env chain.
            #
            # h-pool FIFO gate: hpool slot N+bufs reuses slot N, so a unit
            # may allocate its k h-tiles only when the tiles (bufs-k) back in
            # allocation order are retired (their last reader is emitted).
            LAG = 2
            HBUFS = 22               # must match hpool bufs
            next_stage = {e: 0 for e in range(ENVS)}
            emitted_at = {}
            state = {"n": 0}
            unlocked = {"l1": False, "l2": False, "l3": False, "l4": False}
            l4_done = [False] * NPAIR
            h_retired = []           # per h-allocation: last reader emitted?
            h_idx = {}               # (env, layer) -> [alloc indices]

            state_h = {"unret": 0}

            def h_can_alloc(k, reserve):
                if state_h["unret"] + k > HBUFS - reserve:
                    return False
                for i in range(k):
                    back = len(h_retired) + i - HBUFS
                    if back >= 0 and not h_retired[back]:
                        return False
                return True

            def h_alloc(env, layer):
                h_idx.setdefault((env, layer), []).append(len(h_retired))
                h_retired.append(False)
                state_h["unret"] += 1

            def h_retire(env, layer):
                for i in h_idx.pop((env, layer), []):
                    h_retired[i] = True
                    state_h["unret"] -= 1

            def emit_stage(e):
                si = next_stage[e]
                if si == 0:
                    emit_l1_half(e, 0)
                elif si == 1:
                    emit_l1_half(e, 1)
                elif si == 2:
                    emit_l23_half(e, 0, 0)
                elif si == 3:
                    emit_l23_half(e, 0, 1)
                elif si == 4:
                    emit_l23_half(e, 1, 0)
                else:
                    emit_l23_half(e, 1, 1)
                h_alloc(e, si // 2)
                if si == 3:
                    h_retire(e, 0)          # l2b's MMs are h1's last readers
                elif si == 5:
                    h_retire(e, 1)          # l3b's MMs are h2's last readers
                emitted_at[(si, e)] = state["n"]
                state["n"] += 1
                next_stage[e] = si + 1

            def stage_ready(e, force=False):
                si = next_stage[e]
                if si >= 6:
                    return False
                # l1 pairs grow the live set (+2, retired only at l2b);
                # l2/l3 pairs are net zero but need 2 transient slots
                if si == 0:
                    ok = h_can_alloc(2, 4)
                elif si in (2, 4):
                    ok = h_can_alloc(2, 0)
                else:
                    ok = h_can_alloc(1, 0)
                if not ok:
                    return False
                if si <= 1:
                    return unlocked["l1"]
                if si <= 3:
                    if not unlocked["l2"]:
                        return False
                    if si == 2 and not force:
                        return state["n"] - emitted_at[(1, e)] >= LAG
                    return True
                if not unlocked["l3"]:
                    return False
                if si == 4 and not force:
                    return state["n"] - emitted_at[(3, e)] >= LAG
                return True

            def l4_ready(p, force=False):
                if not unlocked["l4"] or l4_done[p]:
                    return False
                a, b = 2 * p, 2 * p + 1
                if next_stage[a] < 6 or next_stage[b] < 6:
                    return False
                return force or state["n"] - emitted_at[(5, b)] >= LAG

            def try_emit_one():
                for p in range(NPAIR):
                    if l4_ready(p):
                        emit_l4_pair(p)
                        h_retire(2 * p, 2)
                        h_retire(2 * p + 1, 2)
                        l4_done[p] = True
                        return True
                best = None
                for e in range(ENVS):
                    if stage_ready(e):
                        if best is None or next_stage[e] > next_stage[best]:
                            best = e
                if best is None:
                    return False
                emit_stage(best)
                return True

            # ---------------- fused emission ----------------
            # phase 1: delta groups at DMA pace on their own PSUM pool,
            # MLP interleaved on 2-bank [128,1024] drains. Each group's 4
            # regather DMAs are emitted right behind its stores; stage
            # unlocks follow the last relevant regather.
            emit_load(0)
            emit_load(1)
            for jb in range(20):
                if jb + 2 < 20:
                    emit_load(jb + 2)
                emit_group(jb)
                for b in range(4 * jb, 4 * jb + 4):
                    emit_regather_block(b)
                if jb == 1:
                    unlocked["l1"] = True
                elif jb == 10:
                    unlocked["l2"] = True       # W2 done at jb=9
                elif jb == 18:
                    unlocked["l3"] = True       # W3 done at jb=17
                if jb >= 2:
                    for _ in range(2):
                        try_emit_one()
            # phase 2: release delta + small-drain PSUM, full-width ring
            ph1.close()
            with tc.tile_pool(name="psm2", bufs=2, space="PSUM") as psm2:
                cur["pool"] = psm2
                cur["fd"] = 2048
                unlocked["l4"] = True
                while (any(next_stage[e] < 6 for e in range(ENVS))
                       or not all(l4_done)):
                    if not try_emit_one():
                        forced = False
                        for p in range(NPAIR):
                            if l4_ready(p, force=True):
                                emit_l4_pair(p)
                                h_retire(2 * p, 2)
                                h_retire(2 * p + 1, 2)
                                l4_done[p] = True
                                forced = True
                                break
                        if not forced:
                            for e in range(ENVS):
                                if stage_ready(e, force=True):
                                    emit_stage(e)
                                    forced = True
                                    break
                        if not forced:
                            raise AssertionError("emission worklist stuck")

    nc.compile()
    _NC_CACHE["nc"] = nc
    return nc


# --------------------------------------------------------------------------
# entry point
# --------------------------------------------------------------------------
def kernel(t, y, ctx, W1, b1, W2, b2, W3, b3, W4, b4, H, G, beta):
    global LAST_RESULTS
    y = np.asarray(y, np.float32)
    ctx = np.asarray(ctx, np.float32)
    H = np.asarray(H, np.float32)
    G = np.asarray(G, np.float32)
    beta = np.asarray(beta, np.float32)
    W = [np.asarray(w, np.float32) for w in (W1, W2, W3, W4)]
    b = [np.asarray(x, np.float32) for x in (b1, b2, b3, b4)]

    in_maps, host_term = _prep_inputs(y, ctx, W, b, H, G, beta)
    nc = _build_nc()
    res = run_bass_kernel_spmd(
        nc, in_maps, list(range(N_CORES)),
        trace=TRACE, trace_cores=None)
    LAST_RESULTS = res

    total = np.zeros((ENVS, DATA, NPTS), np.float32)
    for e in range(N_CORES):
        total += res.results[e]["out"]
    total += host_term[:, :, None]
    return np.ascontiguousarray(total.transpose(0, 2, 1))


def measure_exec_ns(inputs, iters=64, warmup=4):
    """Steady-state per-execution time of the compiled NEFF on 8 cores.

    Used by test.py only; the grading path never calls this.
    """
    import time
    import jax
    from jax.sharding import Mesh, PartitionSpec, NamedSharding
    from jax.experimental.shard_map import shard_map
    from concourse import bass2jax, mybir as _mybir

    y = np.asarray(inputs["y"], np.float32)
    ctx = np.asarray(inputs["ctx"], np.float32)
    H = np.asarray(inputs["H"], np.float32)
    G = np.asarray(inputs["G"], np.float32)
    beta = np.asarray(inputs["beta"], np.float32)
    W = [np.asarray(inputs[k], np.float32) for k in ("W1", "W2", "W3", "W4")]
    b = [np.asarray(inputs[k], np.float32) for k in ("b1", "b2", "b3", "b4")]
    in_maps, _ = _prep_inputs(y, ctx, W, b, H, G, beta)
    nc = _build_nc()

    bass2jax.install_neuronx_cc_hook()
    partition_name = nc.partition_id_tensor.name if nc.partition_id_tensor else None
    in_names, out_names, out_avals, zero_outs = [], [], [], []
    for alloc in nc.m.functions[0].allocations:
        if not isinstance(alloc, _mybir.MemoryLocationSet):
            continue
        name = alloc.memorylocations[0].name
        if alloc.kind == "ExternalInput":
            if name != partition_name:
                in_names.append(name)
        elif alloc.kind == "ExternalOutput":
            shape = tuple(alloc.tensor_shape)
            dtype = _mybir.dt.np(alloc.dtype)
            out_names.append(name)
            out_avals.append(jax.core.ShapedArray(shape, dtype))
            zero_outs.append(np.zeros(shape, dtype))
    n_params = len(in_names)
    all_in_names = in_names + out_names
    if partition_name is not None:
        all_in_names.append(partition_name)

    def _body(*args):
        operands = list(args)
        if partition_name is not None:
            operands.append(bass2jax.partition_id_tensor())
        outs = bass2jax._bass_exec_p.bind(
            *operands,
            out_avals=tuple(out_avals),
            in_names=tuple(all_in_names),
            out_names=tuple(out_names),
            lowering_input_output_aliases=(),
            sim_require_finite=True,
            sim_require_nnan=True,
            nc=nc,
        )
        return tuple(outs)

    devices = jax.devices()[:N_CORES]
    mesh = Mesh(np.asarray(devices), ("core",))
    nspec = NamedSharding(mesh, PartitionSpec("core"))
    n_all = n_params + len(out_names)
    sharded = jax.jit(
        shard_map(_body, mesh=mesh,
                  in_specs=(PartitionSpec("core"),) * n_all,
                  out_specs=(PartitionSpec("core"),) * len(out_names),
                  check_rep=False),
        keep_unused=True)

    concat_in = [
        np.concatenate([np.asarray(in_maps[c][k]) for c in range(N_CORES)], axis=0)
        for k in in_names
    ] + [np.zeros((N_CORES * z.shape[0], *z.shape[1:]), z.dtype) for z in zero_outs]
    dev_in = [jax.device_put(a, nspec) for a in concat_in]

    for _ in range(warmup):
        outs = sharded(*dev_in)
    jax.block_until_ready(outs)

    t0 = time.perf_counter()
    for _ in range(iters):
        outs = sharded(*dev_in)
    jax.block_until_ready(outs)
    t1 = time.perf_counter()
    per_call = (t1 - t0) / iters

    return {"pipelined_ns": per_call * 1e9}


if __name__ == "__main__":
    _build_nc()
    print("IR build OK")
